# revision 31
# baseline (speedup 1.0000x reference)
"""Trainium2 Bass kernel for the supervised-contrastive loss (nn_KCL_69784628626020).

Strategy (8 NeuronCores, SPMD), v4:
  - Shard anchors (rows of q, k, y) across cores: 1024 rows/core.
  - Each core computes its [1024, 8192] slab of S = q_loc @ q_full^T in
    fp8-e4m3 (DoubleRowSwInterleave: K=256/instr) on the tensor engine.
    The PE queue has NO gating dependencies, so it streams at the 2.4 GHz
    DVFS pstate.  End-to-end fp8 numerics verified: rel err ~3e-4.
  - exp tiles ewu = exp(S/TAU) in bf16 (ACT); the diagonal is zeroed
    in place by one small fused DVE op on the 128-wide strip that can
    contain it (host-provided per-(b,s) selector keeps the program SPMD).
  - P_i = sum_{y_j==y_i, j!=i} ewu_ij  : one fused DVE op per tile (accum).
  - T_j = sum_{i!=j} w_i ewu_ij (w = 1/count, symmetric S): computed as
    WEIGHTED COLUMN SUMS on the tensor engine - matmul with lhsT = the
    local rows' bf16 weights - then ONE ReduceScatter adds the 8 cores'
    column partials and hands each core exactly its rows' totals.
  - Global class counts via a histogram: one-hot rows (DVE) -> ones-matmul
    (PE) -> AllReduce (4 KB); c_i = onehot . h (fused DVE op).  A dummy
    warmup collective absorbs the CC-core startup latency.
  - den_i = log(T_i - w_i P_i),  num_i = log(kpos_i + P_i),
    loss_i = (den_i - num_i) / (c_i - 1 + K).
  - kpos_i = sum_k exp(q_i . k_ik / TAU): fp8 block product q_b @ k_b^T on
    PE, exp on ACT, one fused mask-and-row-reduce on DVE
    (mask[p,c] = c//K == p extracts the generalized diagonal).
  - Final mean: per-core partial via a ones-matmul partition reduction;
    host adds the 8 partials (the unshard step).
"""

import numpy as np
from contextlib import ExitStack

import concourse.bass as bass
import concourse.bacc as bacc
import concourse.tile as tile
from concourse import mybir
from concourse.bass_utils import run_bass_kernel_spmd
import ml_dtypes

F32 = mybir.dt.float32
F16 = mybir.dt.float16
BF16 = mybir.dt.bfloat16
FP8 = mybir.dt.float8e4
FP8_NP = ml_dtypes.float8_e4m3

TAU = 0.07
NCORES = 8
C = 1024          # padded class count (labels < 1000)


class Cfg:
    def __init__(self, N=8192, D=512, KP=8, TW=1024, SLAG=5):
        self.N = N            # total rows (anchors)
        self.D = D            # feature dim
        self.KP = KP          # external positives per anchor
        self.TW = TW          # column tile width
        self.NL = N // NCORES     # rows per core
        self.NB = self.NL // 128  # row blocks per core
        self.NS = N // TW         # column tiles
        self.KC = D // 128        # contraction chunks
        self.KC2 = self.KC // 2   # fp8 pair chunks
        assert self.NL % 128 == 0 and N % TW == 0 and D % 256 == 0
        assert TW % 512 == 0
        self.NCH = TW // 512      # 512-wide matmul chunks per column tile
        self.SLAG = min(SLAG, self.NS - 1)  # column-sum burst lag (in s)
        self.SW = 128 if self.NL == TW else TW
        self.KW = KP * 128        # k-path block product width
        assert self.KW % 512 == 0
        self.NCHK = self.KW // 512


def build_bass(cfg: Cfg):
    N, D, KP, TW = cfg.N, cfg.D, cfg.KP, cfg.TW
    NL, NB, NS, KC2 = cfg.NL, cfg.NB, cfg.NS, cfg.KC2
    NCH, SW, KW, NCHK, SLAG = cfg.NCH, cfg.SW, cfg.KW, cfg.NCHK, cfg.SLAG
    DR = mybir.MatmulPerfMode.DoubleRowSwInterleave

    nc = bacc.Bacc("TRN2", target_bir_lowering=False, debug=False,
                   num_devices=NCORES)

    # ---- kernel I/O -------------------------------------------------------
    qT_d = nc.dram_tensor("qT", [KC2, 128, 2 * N], FP8, kind="ExternalInput")
    qTl_d = nc.dram_tensor("qTl", [KC2, 128, 2 * NL], FP8, kind="ExternalInput")
    kTl_d = nc.dram_tensor("kTl", [KC2, 128, 2 * NL * KP], FP8, kind="ExternalInput")
    ybc_d = nc.dram_tensor("ybc", [128, N], F16, kind="ExternalInput")
    yrow_d = nc.dram_tensor("yrow", [128, NB], F32, kind="ExternalInput")
    clsid_d = nc.dram_tensor("clsid", [128, C], F16, kind="ExternalInput")
    colid_d = nc.dram_tensor("colid", [128, SW], F16, kind="ExternalInput")
    strip_d = nc.dram_tensor("strip", [128, NB * NS], F32, kind="ExternalInput")
    kmask_d = nc.dram_tensor("kmask", [128, KW], F16, kind="ExternalInput")
    out_d = nc.dram_tensor("out", [1, 1], F32, kind="ExternalOutput")

    with tile.TileContext(nc) as tc, ExitStack() as ctx:
        const = ctx.enter_context(tc.tile_pool(name="const", bufs=1))
        rh_pool = ctx.enter_context(tc.tile_pool(name="rh", bufs=2 * KC2))
        psum_pool = ctx.enter_context(tc.tile_pool(name="ps", bufs=2, space="PSUM"))
        psk_pool = ctx.enter_context(tc.tile_pool(name="psk", bufs=1, space="PSUM"))
        pst_pool = ctx.enter_context(tc.tile_pool(name="pst", bufs=1, space="PSUM"))
        ew_pool = ctx.enter_context(
            tc.tile_pool(name="ew", bufs=min((SLAG + 1) * NB - 2, NB * NS + 1)
                         if NB > 1 else (SLAG + 1) * NB + 2))
        t2s_pool = ctx.enter_context(tc.tile_pool(name="t2s", bufs=2))
        oh_pool = ctx.enter_context(tc.tile_pool(name="oh", bufs=2))
        k_pool = ctx.enter_context(tc.tile_pool(name="kp", bufs=KC2))
        ewk_pool = ctx.enter_context(tc.tile_pool(name="ewk", bufs=1))
        ks_pool = ctx.enter_context(tc.tile_pool(name="ks", bufs=1))
        dram = ctx.enter_context(tc.tile_pool(name="dram", bufs=1, space="DRAM"))

        # ---- resident constants (small OH-feeding tensors first) ---------
        yrow = const.tile([128, NB], F32, tag="yrow")
        nc.sync.dma_start(yrow[:, :], yrow_d[:, :])
        clsid = const.tile([128, C], F16, tag="clsid")
        nc.sync.dma_start(clsid[:, :], clsid_d[:, :])
        colid = const.tile([128, SW], F16, tag="colid")
        nc.sync.dma_start(colid[:, :], colid_d[:, :])
        strip = const.tile([128, NB * NS], F32, tag="strip")
        nc.sync.dma_start(strip[:, :], strip_d[:, :])
        qtl = [const.tile([128, 2 * NL], FP8, tag=f"qtl{c}", name=f"qtl{c}")
               for c in range(KC2)]
        for c in range(KC2):
            nc.sync.dma_start(qtl[c][:, :], qTl_d[c, :, :])
        ybc = const.tile([128, N], F16, tag="ybc")
        nc.sync.dma_start(ybc[:, :], ybc_d[:, :])
        kmask = const.tile([128, KW], F16, tag="kmask")
        nc.sync.dma_start(kmask[:, :], kmask_d[:, :])

        ones_col = const.tile([128, 1], F32, tag="ones_col")
        nc.vector.memset(ones_col[:, :], 1.0)
        ones_colb = const.tile([128, 1], BF16, tag="ones_colb")
        nc.vector.memset(ones_colb[:, :], 1.0)

        # accumulator slots
        bslt = const.tile([128, NB * NS], F32, tag="bslt")   # P (excl diag)
        kpos = const.tile([128, NB], F32, tag="kpos")
        cloc = const.tile([128, NB], F32, tag="cloc")
        winv = const.tile([128, NB], F32, tag="winv")
        winvb = const.tile([128, NB], BF16, tag="winvb")
        hbc = const.tile([128, C], F32, tag="hbc")

        # ---- warmup collective (absorbs CC startup latency) --------------
        wusb = const.tile([1, 8], F32, tag="wusb")
        nc.vector.memset(wusb[:, :], 0.0)
        wu_in = dram.tile([1, 8], F32)
        wu_out = dram.tile([1, 8], F32, addr_space="Shared")
        nc.sync.dma_start(wu_in[:, :], wusb[0:1, :])
        nc.gpsimd.collective_compute(
            "AllReduce", mybir.AluOpType.add,
            ins=[wu_in[:, :].opt()], outs=[wu_out[:, :].opt()],
            replica_groups=[list(range(NCORES))],
        )

        # ---- histogram path ----------------------------------------------
        ohrow = [const.tile([128, C], BF16, tag=f"oh{b}", name=f"oh{b}")
                 for b in range(NB)]
        for b in range(NB):
            nc.vector.tensor_scalar(
                ohrow[b][:, :], clsid[:, :], yrow[:, b:b + 1], None,
                op0=mybir.AluOpType.is_equal)

        hsb = const.tile([1, C], F32, tag="hsb")
        hpart_d = dram.tile([1, C], F32)
        hall_d = dram.tile([1, C], F32, addr_space="Shared")

        def emit_hist_mm():
            hps = psk_pool.tile([128, C], F32, tag="psk")
            for b in range(NB):
                for ch in range(C // 512):
                    nc.tensor.matmul(
                        hps[0:1, ch * 512:(ch + 1) * 512],
                        ones_colb[:, :],
                        ohrow[b][:, ch * 512:(ch + 1) * 512],
                        start=(b == 0), stop=(b == NB - 1))
            nc.scalar.copy(hsb[0:1, :], hps[0:1, :])
            nc.sync.dma_start(hpart_d[:, :], hsb[0:1, :])
            nc.gpsimd.collective_compute(
                "AllReduce", mybir.AluOpType.add,
                ins=[hpart_d[:, :].opt()],
                outs=[hall_d[:, :].opt()],
                replica_groups=[list(range(NCORES))],
            )
            nc.sync.dma_start(hbc[:, :], hall_d[:, :].broadcast_to([128, C]))

        # ---- T column-sum staging / exchange -----------------------------
        Tstage_d = dram.tile([NS, TW], F32)
        Trs_d = dram.tile([1, NL], F32)
        tsb = const.tile([1, NS * TW], F32, tag="tsb")
        ewus = {}   # (b, s) -> ewu tile pending column sums

        def emit_colsum_burst(s):
            pst = pst_pool.tile([128, TW], F32, tag="pst")
            for ch in range(NCH):
                for b in range(NB):
                    nc.tensor.matmul(
                        pst[0:1, ch * 512:(ch + 1) * 512],
                        winvb[:, b:b + 1],
                        ewus[(b, s)][:, ch * 512:(ch + 1) * 512],
                        start=(b == 0), stop=(b == NB - 1))
            nc.scalar.copy(tsb[0:1, s * TW:(s + 1) * TW], pst[0:1, :])
            nc.sync.dma_start(Tstage_d[s:s + 1, :], tsb[0:1, s * TW:(s + 1) * TW])
            for b in range(NB):
                del ewus[(b, s)]

        # ---- k-path (fp8 block product) ----------------------------------
        kpath_after_s = {(b * NS) // NB: b for b in range(NB)}
        assert len(kpath_after_s) == NB, "need NS >= NB for k-path interleave"

        def emit_kpath(b):
            ktl = [k_pool.tile([128, 2 * KW], FP8, tag="ktl", name=f"ktl{b}_{c2}")
                   for c2 in range(KC2)]
            for c in range(KC2):
                nc.sync.dma_start(ktl[c][:, :],
                                  kTl_d[c, :, b * 2 * KW:(b + 1) * 2 * KW])
            psk = psk_pool.tile([128, C], F32, tag="psk")
            for nch in range(NCHK):
                o = psk[:, nch * 512:(nch + 1) * 512]
                for c in range(KC2):
                    nc.tensor.matmul(
                        o,
                        qtl[c][:, 2 * b * 128:2 * (b + 1) * 128],
                        ktl[c][:, 2 * nch * 512:2 * (nch + 1) * 512].rearrange(
                            "p (n two) -> p two n", two=2),
                        start=(c == 0), stop=(c == KC2 - 1), perf_mode=DR)
            ewk = ewk_pool.tile([128, KW], F32, tag="ewk")
            nc.scalar.activation(ewk[:, :], psk[:, :KW],
                                 mybir.ActivationFunctionType.Exp,
                                 scale=float(1.0 / TAU))
            kscr = ks_pool.tile([128, KW], BF16, tag="kscr")
            nc.vector.scalar_tensor_tensor(
                kscr[:, :], kmask[:, :], 1.0, ewk[:, :],
                op0=mybir.AluOpType.mult, op1=mybir.AluOpType.mult,
                accum_out=kpos[:, b:b + 1])

        # ---- main loop ---------------------------------------------------
        TT = NB * NS
        CL0 = min(NB, TT - NB)   # cloc ops spread after tiles CL0..
        hist_done = [False]

        for s in range(NS):
            rhs = [rh_pool.tile([128, 2 * TW], FP8, tag="rh", name=f"rhs{s}_{c2}")
                   for c2 in range(KC2)]
            for c in range(KC2):
                nc.sync.dma_start(rhs[c][:, :],
                                  qT_d[c, :, 2 * s * TW:2 * (s + 1) * TW])
            for b in range(NB):
                k = s * NB + b
                idx = b * NS + s
                ps = psum_pool.tile([128, TW], F32)
                for nch in range(NCH):
                    o = ps[:, nch * 512:(nch + 1) * 512]
                    for c in range(KC2):
                        nc.tensor.matmul(
                            o,
                            qtl[c][:, 2 * b * 128:2 * (b + 1) * 128],
                            rhs[c][:, 2 * nch * 512:2 * (nch + 1) * 512].rearrange(
                                "p (n two) -> p two n", two=2),
                            start=(c == 0), stop=(c == KC2 - 1), perf_mode=DR)
                ewu = ew_pool.tile([128, TW], BF16)
                nc.scalar.activation(ewu[:, :], ps[:, :],
                                     mybir.ActivationFunctionType.Exp,
                                     scale=float(1.0 / TAU))
                # zero the diagonal strip in place (no-op rows: strip == -1)
                coff = b * 128 if SW == 128 else 0
                nc.vector.scalar_tensor_tensor(
                    ewu[:, coff:coff + SW], colid[:, :],
                    strip[:, idx:idx + 1],
                    ewu[:, coff:coff + SW],
                    op0=mybir.AluOpType.not_equal, op1=mybir.AluOpType.mult)
                # P: same-class row-sum (diag already zeroed)
                t2s = t2s_pool.tile([128, TW], BF16, tag="t2s")
                nc.vector.scalar_tensor_tensor(
                    t2s[:, :], ybc[:, s * TW:(s + 1) * TW], yrow[:, b:b + 1],
                    ewu[:, :],
                    op0=mybir.AluOpType.is_equal, op1=mybir.AluOpType.mult,
                    accum_out=bslt[:, idx:idx + 1])
                ewus[(b, s)] = ewu
                if k == 1 and not hist_done[0]:
                    emit_hist_mm()
                    hist_done[0] = True
                if CL0 <= k < CL0 + NB:
                    b2 = k - CL0
                    ohs = oh_pool.tile([128, C], F32, tag="ohs")
                    nc.vector.scalar_tensor_tensor(
                        ohs[:, :], ohrow[b2][:, :], 1.0, hbc[:, :],
                        op0=mybir.AluOpType.mult, op1=mybir.AluOpType.mult,
                        accum_out=cloc[:, b2:b2 + 1])
                    if b2 == NB - 1:
                        nc.vector.reciprocal(winv[:, :], cloc[:, :])
                        nc.vector.tensor_scalar_mul(winvb[:, :], winv[:, :], 1.0)
            if s in kpath_after_s:
                emit_kpath(kpath_after_s[s])
            if s >= SLAG:
                emit_colsum_burst(s - SLAG)
        for s2 in range(NS - SLAG, NS):
            emit_colsum_burst(s2)

        # ---- finalize part 1: everything not needing T -------------------
        Pcol = const.tile([128, NB], F32, tag="Pcol")
        for b in range(NB):
            nc.vector.tensor_reduce(Pcol[:, b:b + 1], bslt[:, b * NS:(b + 1) * NS],
                                    mybir.AxisListType.X, mybir.AluOpType.add)
        wp = const.tile([128, NB], F32, tag="wp")
        nc.vector.tensor_tensor(wp[:, :], Pcol[:, :], winvb[:, :],
                                op=mybir.AluOpType.mult)
        num_in = const.tile([128, NB], F32, tag="num_in")
        nc.vector.tensor_tensor(num_in[:, :], Pcol[:, :], kpos[:, :],
                                op=mybir.AluOpType.add)
        num_l = const.tile([128, NB], F32, tag="num_l")
        nc.scalar.activation(num_l[:, :], num_in[:, :],
                             mybir.ActivationFunctionType.Ln)
        dnm = const.tile([128, NB], F32, tag="dnm")
        nc.vector.tensor_scalar_add(dnm[:, :], cloc[:, :], float(KP - 1))
        dinv = const.tile([128, NB], F32, tag="dinv")
        nc.vector.reciprocal(dinv[:, :], dnm[:, :])

        # ---- exchange: ReduceScatter of T column partials ----------------
        nc.gpsimd.collective_compute(
            "ReduceScatter", mybir.AluOpType.add,
            ins=[Tstage_d[:, :].opt()],
            outs=[Trs_d[:, :].opt()],
            replica_groups=[list(range(NCORES))],
        )
        Trow = const.tile([128, NB], F32, tag="Trow")
        nc.sync.dma_start(
            Trow[:, :],
            Trs_d[:, :].rearrange("o (b p) -> p (o b)", b=NB, p=128))

        # ---- finalize part 2: den side -----------------------------------
        den_in = const.tile([128, NB], F32, tag="den_in")
        nc.vector.tensor_tensor(den_in[:, :], Trow[:, :], wp[:, :],
                                op=mybir.AluOpType.subtract)
        den_l = const.tile([128, NB], F32, tag="den_l")
        nc.scalar.activation(den_l[:, :], den_in[:, :],
                             mybir.ActivationFunctionType.Ln)
        diff = const.tile([128, NB], F32, tag="diff")
        nc.vector.tensor_tensor(diff[:, :], den_l[:, :], num_l[:, :],
                                op=mybir.AluOpType.subtract)
        losscol = const.tile([128, NB], F32, tag="losscol")
        nc.vector.tensor_tensor(losscol[:, :], diff[:, :], dinv[:, :],
                                op=mybir.AluOpType.mult)

        lsum = const.tile([128, 1], F32, tag="lsum")
        nc.vector.tensor_reduce(lsum[:, :], losscol[:, :],
                                mybir.AxisListType.X, mybir.AluOpType.add)
        psf = psk_pool.tile([128, C], F32, tag="psk")
        nc.tensor.matmul(psf[0:1, 0:1], lsum[:, :],
                         ones_col[:, :], start=True, stop=True)
        outsb = const.tile([1, 1], F32, tag="outsb")
        nc.scalar.copy(outsb[0:1, 0:1], psf[0:1, 0:1])
        nc.sync.dma_start(out_d[:, :], outsb[0:1, 0:1])

    nc.compile()
    return nc


# ---------------------------------------------------------------------------
# host-side marshalling
# ---------------------------------------------------------------------------

def _pair_layout(mT):
    """[D, X] -> ifmap pair layout [KC2, 128, 2*X], free = (x, pair) pair-minor."""
    Dd, X = mT.shape
    KC2 = Dd // 256
    a = mT.reshape(KC2, 2, 128, X)          # [c2, i, d', x]
    a = a.transpose(0, 2, 3, 1)             # [c2, d', x, i]
    return np.ascontiguousarray(a.reshape(KC2, 128, 2 * X))


def _swint_layout(mT):
    """[D, X] -> DoubleRowSwInterleave weights layout [KC2, 128, 2*X]."""
    Dd, X = mT.shape
    KC2 = Dd // 256
    NBK = X // 128
    a = mT.reshape(KC2, 2, 128, NBK, 128)    # [c2, u, d', blk, m]
    a = a[:, :, :, :, ::-1]                  # reverse m -> j
    a = a.transpose(0, 2, 3, 4, 1)           # [c2, d', blk, j, u]
    return np.ascontiguousarray(a.reshape(KC2, 128, 2 * X))


def make_inputs(q, k, y, cfg: Cfg):
    """Build the per-core input maps (pure layout/replication marshalling)."""
    N, D, KP, TW = cfg.N, cfg.D, cfg.KP, cfg.TW
    NL, NB, NS, SW = cfg.NL, cfg.NB, cfg.NS, cfg.SW
    q = np.asarray(q, dtype=np.float32)
    k = np.asarray(k, dtype=np.float32)
    y = np.asarray(y)

    q8T = np.ascontiguousarray(q.astype(FP8_NP).T)           # [D, N] fp8
    qT = _pair_layout(q8T)
    ybc = np.broadcast_to(y.astype(np.float16)[None, :], (128, N)).copy()
    clsid = np.broadcast_to(np.arange(C, dtype=np.float16)[None, :], (128, C)).copy()
    colid = np.broadcast_to(np.arange(SW, dtype=np.float16)[None, :], (128, SW)).copy()
    KW = KP * 128
    kmask = (np.arange(KW)[None, :] // KP == np.arange(128)[:, None]).astype(np.float16)

    in_maps = []
    p = np.arange(128)
    for r in range(NCORES):
        rows = slice(r * NL, (r + 1) * NL)
        q8l = q[rows].astype(FP8_NP)
        qTl = _swint_layout(np.ascontiguousarray(q8l.T))
        kloc = k[rows].reshape(NL * KP, D).astype(FP8_NP)
        kTl = _pair_layout(np.ascontiguousarray(kloc.T))
        yrow = np.ascontiguousarray(y[rows].astype(np.float32).reshape(NB, 128).T)
        strip = np.full((128, NB * NS), -1.0, dtype=np.float32)
        for b in range(NB):
            grow = r * NL + b * 128 + p
            for s in range(NS):
                col = grow - s * TW
                coff = b * 128 if SW == 128 else 0
                scol = col - coff
                m = (scol >= 0) & (scol < SW) & (col >= 0) & (col < TW)
                strip[m, b * NS + s] = scol[m]
        in_maps.append({
            "qT": qT, "qTl": qTl, "kTl": kTl,
            "ybc": ybc, "yrow": yrow, "clsid": clsid, "colid": colid,
            "strip": strip, "kmask": kmask,
        })
    return in_maps


_CACHE = {}


def _get_nc(cfg_key):
    if cfg_key not in _CACHE:
        cfg = Cfg()
        _CACHE[cfg_key] = (cfg, build_bass(cfg))
    return _CACHE[cfg_key]


def kernel(q, k, y, trace=False):
    cfg, nc = _get_nc("full")
    in_maps = make_inputs(q, k, y, cfg)
    res = run_bass_kernel_spmd(nc, in_maps, core_ids=list(range(NCORES)),
                               trace=trace)
    total = np.sum([res.results[r]["out"][0, 0] for r in range(NCORES)],
                   dtype=np.float64)
    out = np.asarray(total / cfg.N, dtype=np.float32)
    if trace:
        kernel.last_results = res
    return out


# revision 33
# speedup vs baseline: 1.2997x; 1.2997x over previous
"""Trainium2 Bass kernel for the supervised-contrastive loss (nn_KCL_69784628626020).

Strategy (8 NeuronCores, SPMD), v4:
  - Shard anchors (rows of q, k, y) across cores: 1024 rows/core.
  - Each core computes its [1024, 8192] slab of S = q_loc @ q_full^T in
    fp8-e4m3 (DoubleRowSwInterleave: K=256/instr) on the tensor engine.
    The PE queue has NO gating dependencies, so it streams at the 2.4 GHz
    DVFS pstate.  End-to-end fp8 numerics verified: rel err ~3e-4.
  - exp tiles ewu = exp(S/TAU) in bf16 (ACT); the diagonal is zeroed
    in place by one small fused DVE op on the 128-wide strip that can
    contain it (host-provided per-(b,s) selector keeps the program SPMD).
  - P_i = sum_{y_j==y_i, j!=i} ewu_ij  : one fused DVE op per tile (accum).
  - T_j = sum_{i!=j} w_i ewu_ij (w = 1/count, symmetric S): computed as
    WEIGHTED COLUMN SUMS on the tensor engine - matmul with lhsT = the
    local rows' bf16 weights - then ONE ReduceScatter adds the 8 cores'
    column partials and hands each core exactly its rows' totals.
  - Global class counts via a histogram: one-hot rows (DVE) -> ones-matmul
    (PE) -> AllReduce (4 KB); c_i = onehot . h (fused DVE op).  A dummy
    warmup collective absorbs the CC-core startup latency.
  - den_i = log(T_i - w_i P_i),  num_i = log(kpos_i + P_i),
    loss_i = (den_i - num_i) / (c_i - 1 + K).
  - kpos_i = sum_k exp(q_i . k_ik / TAU): fp8 block product q_b @ k_b^T on
    PE, exp on ACT, one fused mask-and-row-reduce on DVE
    (mask[p,c] = c//K == p extracts the generalized diagonal).
  - Final mean: per-core partial via a ones-matmul partition reduction;
    host adds the 8 partials (the unshard step).
"""

import numpy as np
from contextlib import ExitStack

import concourse.bass as bass
import concourse.bacc as bacc
import concourse.tile as tile
from concourse import mybir
from concourse.bass_utils import run_bass_kernel_spmd
import ml_dtypes

F32 = mybir.dt.float32
F16 = mybir.dt.float16
BF16 = mybir.dt.bfloat16
FP8 = mybir.dt.float8e4
FP8_NP = ml_dtypes.float8_e4m3

TAU = 0.07
NCORES = 8
C = 1024          # padded class count (labels < 1000)


class Cfg:
    def __init__(self, N=8192, D=512, KP=8, TW=1024, SLAG=5):
        self.N = N            # total rows (anchors)
        self.D = D            # feature dim
        self.KP = KP          # external positives per anchor
        self.TW = TW          # column tile width
        self.NL = N // NCORES     # rows per core
        self.NB = self.NL // 128  # row blocks per core
        self.NS = N // TW         # column tiles
        self.KC = D // 128        # contraction chunks
        self.KC2 = self.KC // 2   # fp8 pair chunks
        assert self.NL % 128 == 0 and N % TW == 0 and D % 256 == 0
        assert TW % 512 == 0
        self.NCH = TW // 512      # 512-wide matmul chunks per column tile
        self.SLAG = min(SLAG, self.NS - 1)  # column-sum burst lag (in s)
        self.SW = 128 if self.NL == TW else TW
        self.KW = KP * 128        # k-path block product width
        assert self.KW % 512 == 0
        self.NCHK = self.KW // 512


def build_bass(cfg: Cfg):
    N, D, KP, TW = cfg.N, cfg.D, cfg.KP, cfg.TW
    NL, NB, NS, KC2 = cfg.NL, cfg.NB, cfg.NS, cfg.KC2
    NCH, SW, KW, NCHK, SLAG = cfg.NCH, cfg.SW, cfg.KW, cfg.NCHK, cfg.SLAG
    DR = mybir.MatmulPerfMode.DoubleRowSwInterleave

    nc = bacc.Bacc("TRN2", target_bir_lowering=False, debug=False,
                   num_devices=NCORES)

    # ---- kernel I/O -------------------------------------------------------
    qT_d = nc.dram_tensor("qT", [KC2, 128, 2 * N], FP8, kind="ExternalInput")
    qTl_d = nc.dram_tensor("qTl", [KC2, 128, 2 * NL], FP8, kind="ExternalInput")
    kTl_d = nc.dram_tensor("kTl", [KC2, 128, 2 * NL * KP], FP8, kind="ExternalInput")
    ybc_d = nc.dram_tensor("ybc", [128, N], F16, kind="ExternalInput")
    yrow_d = nc.dram_tensor("yrow", [128, NB], F32, kind="ExternalInput")
    clsid_d = nc.dram_tensor("clsid", [128, C], F16, kind="ExternalInput")
    colid_d = nc.dram_tensor("colid", [128, SW], F16, kind="ExternalInput")
    strip_d = nc.dram_tensor("strip", [128, NB * NS], F32, kind="ExternalInput")
    kmask_d = nc.dram_tensor("kmask", [128, KW], F16, kind="ExternalInput")
    out_d = nc.dram_tensor("out", [1, 1], F32, kind="ExternalOutput")

    with tile.TileContext(nc) as tc, ExitStack() as ctx:
        const = ctx.enter_context(tc.tile_pool(name="const", bufs=1))
        rh_pool = ctx.enter_context(tc.tile_pool(name="rh", bufs=2 * KC2))
        psum_pool = ctx.enter_context(tc.tile_pool(name="ps", bufs=3, space="PSUM"))
        psk_pool = ctx.enter_context(tc.tile_pool(name="psk", bufs=1, space="PSUM"))
        ew_pool = ctx.enter_context(
            tc.tile_pool(name="ew", bufs=min((SLAG + 1) * NB - 2, NB * NS + 1)
                         if NB > 1 else (SLAG + 1) * NB + 2))
        t2s_pool = ctx.enter_context(tc.tile_pool(name="t2s", bufs=2))
        oh_pool = ctx.enter_context(tc.tile_pool(name="oh", bufs=2))
        k_pool = ctx.enter_context(tc.tile_pool(name="kp", bufs=KC2))
        ewk_pool = ctx.enter_context(tc.tile_pool(name="ewk", bufs=1))
        ks_pool = ctx.enter_context(tc.tile_pool(name="ks", bufs=1))
        dram = ctx.enter_context(tc.tile_pool(name="dram", bufs=1, space="DRAM"))

        # ---- resident constants (small OH-feeding tensors first) ---------
        yrow = const.tile([128, NB], F32, tag="yrow")
        nc.sync.dma_start(yrow[:, :], yrow_d[:, :])
        clsid = const.tile([128, C], F16, tag="clsid")
        nc.sync.dma_start(clsid[:, :], clsid_d[:, :])
        colid = const.tile([128, SW], F16, tag="colid")
        nc.sync.dma_start(colid[:, :], colid_d[:, :])
        strip = const.tile([128, NB * NS], F32, tag="strip")
        nc.sync.dma_start(strip[:, :], strip_d[:, :])
        qtl = [const.tile([128, 2 * NL], FP8, tag=f"qtl{c}", name=f"qtl{c}")
               for c in range(KC2)]
        for c in range(KC2):
            nc.sync.dma_start(qtl[c][:, :], qTl_d[c, :, :])
        ybc = const.tile([128, N], F16, tag="ybc")
        kmask = const.tile([128, KW], F16, tag="kmask")

        ones_col = const.tile([128, 1], F32, tag="ones_col")
        nc.vector.memset(ones_col[:, :], 1.0)
        ones_colb = const.tile([128, 1], BF16, tag="ones_colb")
        nc.vector.memset(ones_colb[:, :], 1.0)

        # accumulator slots
        bslt = const.tile([128, NB * NS], F32, tag="bslt")   # P (excl diag)
        kpos = const.tile([128, NB], F32, tag="kpos")
        cloc = const.tile([128, NB], F32, tag="cloc")
        winv = const.tile([128, NB], F32, tag="winv")
        winvb = const.tile([128, NB], BF16, tag="winvb")
        hbc = const.tile([128, C], F32, tag="hbc")

        # ---- histogram path ----------------------------------------------
        ohrow = [const.tile([128, C], BF16, tag=f"oh{b}", name=f"oh{b}")
                 for b in range(NB)]
        for b in range(NB):
            nc.vector.tensor_scalar(
                ohrow[b][:, :], clsid[:, :], yrow[:, b:b + 1], None,
                op0=mybir.AluOpType.is_equal)

        hsb = const.tile([1, C], F32, tag="hsb")
        hpart_d = dram.tile([1, C], F32)
        hall_d = dram.tile([1, C], F32, addr_space="Shared")

        def emit_hist_mm():
            hps = psk_pool.tile([128, C], F32, tag="psk")
            for b in range(NB):
                for ch in range(C // 512):
                    nc.tensor.matmul(
                        hps[0:1, ch * 512:(ch + 1) * 512],
                        ones_colb[:, :],
                        ohrow[b][:, ch * 512:(ch + 1) * 512],
                        start=(b == 0), stop=(b == NB - 1))
            nc.scalar.copy(hsb[0:1, :], hps[0:1, :])
            nc.sync.dma_start(hpart_d[:, :], hsb[0:1, :])
            nc.gpsimd.collective_compute(
                "AllReduce", mybir.AluOpType.add,
                ins=[hpart_d[:, :].opt()],
                outs=[hall_d[:, :].opt()],
                replica_groups=[list(range(NCORES))],
            )
            nc.sync.dma_start(hbc[:, :], hall_d[:, :].broadcast_to([128, C]))

        # ---- T column-sum staging / exchange -----------------------------
        Tstage_d = dram.tile([NS, TW], F32)
        Trs_d = dram.tile([1, NL], F32)
        tsb = const.tile([1, NS * TW], F32, tag="tsb")
        ewus = {}   # (b, s) -> ewu tile pending column sums

        def emit_colsum_burst(s):
            pst = psum_pool.tile([128, TW], F32, tag="ps")
            for ch in range(NCH):
                for b in range(NB):
                    nc.tensor.matmul(
                        pst[0:1, ch * 512:(ch + 1) * 512],
                        winvb[:, b:b + 1],
                        ewus[(b, s)][:, ch * 512:(ch + 1) * 512],
                        start=(b == 0), stop=(b == NB - 1))
            nc.scalar.copy(tsb[0:1, s * TW:(s + 1) * TW], pst[0:1, :])
            nc.sync.dma_start(Tstage_d[s:s + 1, :], tsb[0:1, s * TW:(s + 1) * TW])
            for b in range(NB):
                del ewus[(b, s)]

        # ---- k-path (fp8 block product) ----------------------------------
        kpath_after_s = {(b * NS) // NB: b for b in range(NB)}
        assert len(kpath_after_s) == NB, "need NS >= NB for k-path interleave"

        def emit_kpath(b):
            ktl = [k_pool.tile([128, 2 * KW], FP8, tag="ktl", name=f"ktl{b}_{c2}")
                   for c2 in range(KC2)]
            for c in range(KC2):
                nc.sync.dma_start(ktl[c][:, :],
                                  kTl_d[c, :, b * 2 * KW:(b + 1) * 2 * KW])
            psk = psk_pool.tile([128, C], F32, tag="psk")
            for nch in range(NCHK):
                o = psk[:, nch * 512:(nch + 1) * 512]
                for c in range(KC2):
                    nc.tensor.matmul(
                        o,
                        qtl[c][:, 2 * b * 128:2 * (b + 1) * 128],
                        ktl[c][:, 2 * nch * 512:2 * (nch + 1) * 512].rearrange(
                            "p (n two) -> p two n", two=2),
                        start=(c == 0), stop=(c == KC2 - 1), perf_mode=DR)
            ewk = ewk_pool.tile([128, KW], F32, tag="ewk")
            nc.scalar.activation(ewk[:, :], psk[:, :KW],
                                 mybir.ActivationFunctionType.Exp,
                                 scale=float(1.0 / TAU))
            kscr = ks_pool.tile([128, KW], BF16, tag="kscr")
            nc.vector.scalar_tensor_tensor(
                kscr[:, :], kmask[:, :], 1.0, ewk[:, :],
                op0=mybir.AluOpType.mult, op1=mybir.AluOpType.mult,
                accum_out=kpos[:, b:b + 1])

        # ---- main loop ---------------------------------------------------
        TT = NB * NS
        CL0 = min(2 * NB, TT - NB)   # cloc ops spread after tiles CL0..
        hist_done = [False]

        for s in range(NS):
            rhs = [rh_pool.tile([128, 2 * TW], FP8, tag="rh", name=f"rhs{s}_{c2}")
                   for c2 in range(KC2)]
            for c in range(KC2):
                nc.sync.dma_start(rhs[c][:, :],
                                  qT_d[c, :, 2 * s * TW:2 * (s + 1) * TW])
            if s == 0:
                nc.sync.dma_start(ybc[:, :], ybc_d[:, :])
                nc.sync.dma_start(kmask[:, :], kmask_d[:, :])
            for b in range(NB):
                k = s * NB + b
                idx = b * NS + s
                ps = psum_pool.tile([128, TW], F32, tag="ps")
                for nch in range(NCH):
                    o = ps[:, nch * 512:(nch + 1) * 512]
                    for c in range(KC2):
                        nc.tensor.matmul(
                            o,
                            qtl[c][:, 2 * b * 128:2 * (b + 1) * 128],
                            rhs[c][:, 2 * nch * 512:2 * (nch + 1) * 512].rearrange(
                                "p (n two) -> p two n", two=2),
                            start=(c == 0), stop=(c == KC2 - 1), perf_mode=DR)
                ewu = ew_pool.tile([128, TW], BF16)
                nc.scalar.activation(ewu[:, :], ps[:, :],
                                     mybir.ActivationFunctionType.Exp,
                                     scale=float(1.0 / TAU))
                # zero the diagonal strip in place (no-op rows: strip == -1)
                coff = b * 128 if SW == 128 else 0
                nc.vector.scalar_tensor_tensor(
                    ewu[:, coff:coff + SW], colid[:, :],
                    strip[:, idx:idx + 1],
                    ewu[:, coff:coff + SW],
                    op0=mybir.AluOpType.not_equal, op1=mybir.AluOpType.mult)
                # P: same-class row-sum (diag already zeroed)
                t2s = t2s_pool.tile([128, TW], BF16, tag="t2s")
                nc.vector.scalar_tensor_tensor(
                    t2s[:, :], ybc[:, s * TW:(s + 1) * TW], yrow[:, b:b + 1],
                    ewu[:, :],
                    op0=mybir.AluOpType.is_equal, op1=mybir.AluOpType.mult,
                    accum_out=bslt[:, idx:idx + 1])
                ewus[(b, s)] = ewu
                if k == 0 and not hist_done[0]:
                    emit_hist_mm()
                    hist_done[0] = True
                if CL0 <= k < CL0 + NB:
                    b2 = k - CL0
                    ohs = oh_pool.tile([128, C], F32, tag="ohs")
                    nc.vector.scalar_tensor_tensor(
                        ohs[:, :], ohrow[b2][:, :], 1.0, hbc[:, :],
                        op0=mybir.AluOpType.mult, op1=mybir.AluOpType.mult,
                        accum_out=cloc[:, b2:b2 + 1])
                    if b2 == NB - 1:
                        nc.vector.reciprocal(winv[:, :], cloc[:, :])
                        nc.vector.tensor_scalar_mul(winvb[:, :], winv[:, :], 1.0)
            if s in kpath_after_s:
                emit_kpath(kpath_after_s[s])
            if s >= SLAG:
                emit_colsum_burst(s - SLAG)
        for s2 in range(NS - SLAG, NS):
            emit_colsum_burst(s2)

        # ---- finalize part 1: everything not needing T -------------------
        Pcol = const.tile([128, NB], F32, tag="Pcol")
        for b in range(NB):
            nc.vector.tensor_reduce(Pcol[:, b:b + 1], bslt[:, b * NS:(b + 1) * NS],
                                    mybir.AxisListType.X, mybir.AluOpType.add)
        wp = const.tile([128, NB], F32, tag="wp")
        nc.vector.tensor_tensor(wp[:, :], Pcol[:, :], winvb[:, :],
                                op=mybir.AluOpType.mult)
        num_in = const.tile([128, NB], F32, tag="num_in")
        nc.vector.tensor_tensor(num_in[:, :], Pcol[:, :], kpos[:, :],
                                op=mybir.AluOpType.add)
        num_l = const.tile([128, NB], F32, tag="num_l")
        nc.scalar.activation(num_l[:, :], num_in[:, :],
                             mybir.ActivationFunctionType.Ln)
        dnm = const.tile([128, NB], F32, tag="dnm")
        nc.vector.tensor_scalar_add(dnm[:, :], cloc[:, :], float(KP - 1))
        dinv = const.tile([128, NB], F32, tag="dinv")
        nc.vector.reciprocal(dinv[:, :], dnm[:, :])

        # ---- exchange: ReduceScatter of T column partials ----------------
        nc.gpsimd.collective_compute(
            "ReduceScatter", mybir.AluOpType.add,
            ins=[Tstage_d[:, :].opt()],
            outs=[Trs_d[:, :].opt()],
            replica_groups=[list(range(NCORES))],
        )
        Trow = const.tile([128, NB], F32, tag="Trow")
        nc.sync.dma_start(
            Trow[:, :],
            Trs_d[:, :].rearrange("o (b p) -> p (o b)", b=NB, p=128))

        # ---- finalize part 2: den side -----------------------------------
        den_in = const.tile([128, NB], F32, tag="den_in")
        nc.vector.tensor_tensor(den_in[:, :], Trow[:, :], wp[:, :],
                                op=mybir.AluOpType.subtract)
        den_l = const.tile([128, NB], F32, tag="den_l")
        nc.scalar.activation(den_l[:, :], den_in[:, :],
                             mybir.ActivationFunctionType.Ln)
        diff = const.tile([128, NB], F32, tag="diff")
        nc.vector.tensor_tensor(diff[:, :], den_l[:, :], num_l[:, :],
                                op=mybir.AluOpType.subtract)
        losscol = const.tile([128, NB], F32, tag="losscol")
        nc.vector.tensor_tensor(losscol[:, :], diff[:, :], dinv[:, :],
                                op=mybir.AluOpType.mult)

        lsum = const.tile([128, 1], F32, tag="lsum")
        nc.vector.tensor_reduce(lsum[:, :], losscol[:, :],
                                mybir.AxisListType.X, mybir.AluOpType.add)
        psf = psk_pool.tile([128, C], F32, tag="psk")
        nc.tensor.matmul(psf[0:1, 0:1], lsum[:, :],
                         ones_col[:, :], start=True, stop=True)
        outsb = const.tile([1, 1], F32, tag="outsb")
        nc.scalar.copy(outsb[0:1, 0:1], psf[0:1, 0:1])
        nc.sync.dma_start(out_d[:, :], outsb[0:1, 0:1])

    nc.compile()
    return nc


# ---------------------------------------------------------------------------
# host-side marshalling
# ---------------------------------------------------------------------------

def _pair_layout(mT):
    """[D, X] -> ifmap pair layout [KC2, 128, 2*X], free = (x, pair) pair-minor."""
    Dd, X = mT.shape
    KC2 = Dd // 256
    a = mT.reshape(KC2, 2, 128, X)          # [c2, i, d', x]
    a = a.transpose(0, 2, 3, 1)             # [c2, d', x, i]
    return np.ascontiguousarray(a.reshape(KC2, 128, 2 * X))


def _swint_layout(mT):
    """[D, X] -> DoubleRowSwInterleave weights layout [KC2, 128, 2*X]."""
    Dd, X = mT.shape
    KC2 = Dd // 256
    NBK = X // 128
    a = mT.reshape(KC2, 2, 128, NBK, 128)    # [c2, u, d', blk, m]
    a = a[:, :, :, :, ::-1]                  # reverse m -> j
    a = a.transpose(0, 2, 3, 4, 1)           # [c2, d', blk, j, u]
    return np.ascontiguousarray(a.reshape(KC2, 128, 2 * X))


def make_inputs(q, k, y, cfg: Cfg):
    """Build the per-core input maps (pure layout/replication marshalling)."""
    N, D, KP, TW = cfg.N, cfg.D, cfg.KP, cfg.TW
    NL, NB, NS, SW = cfg.NL, cfg.NB, cfg.NS, cfg.SW
    q = np.asarray(q, dtype=np.float32)
    k = np.asarray(k, dtype=np.float32)
    y = np.asarray(y)

    q8T = np.ascontiguousarray(q.astype(FP8_NP).T)           # [D, N] fp8
    qT = _pair_layout(q8T)
    ybc = np.broadcast_to(y.astype(np.float16)[None, :], (128, N)).copy()
    clsid = np.broadcast_to(np.arange(C, dtype=np.float16)[None, :], (128, C)).copy()
    colid = np.broadcast_to(np.arange(SW, dtype=np.float16)[None, :], (128, SW)).copy()
    KW = KP * 128
    kmask = (np.arange(KW)[None, :] // KP == np.arange(128)[:, None]).astype(np.float16)

    in_maps = []
    p = np.arange(128)
    for r in range(NCORES):
        rows = slice(r * NL, (r + 1) * NL)
        q8l = q[rows].astype(FP8_NP)
        qTl = _swint_layout(np.ascontiguousarray(q8l.T))
        kloc = k[rows].reshape(NL * KP, D).astype(FP8_NP)
        kTl = _pair_layout(np.ascontiguousarray(kloc.T))
        yrow = np.ascontiguousarray(y[rows].astype(np.float32).reshape(NB, 128).T)
        strip = np.full((128, NB * NS), -1.0, dtype=np.float32)
        for b in range(NB):
            grow = r * NL + b * 128 + p
            for s in range(NS):
                col = grow - s * TW
                coff = b * 128 if SW == 128 else 0
                scol = col - coff
                m = (scol >= 0) & (scol < SW) & (col >= 0) & (col < TW)
                strip[m, b * NS + s] = scol[m]
        in_maps.append({
            "qT": qT, "qTl": qTl, "kTl": kTl,
            "ybc": ybc, "yrow": yrow, "clsid": clsid, "colid": colid,
            "strip": strip, "kmask": kmask,
        })
    return in_maps


_CACHE = {}


def _get_nc(cfg_key):
    if cfg_key not in _CACHE:
        cfg = Cfg()
        _CACHE[cfg_key] = (cfg, build_bass(cfg))
    return _CACHE[cfg_key]


def kernel(q, k, y, trace=False):
    cfg, nc = _get_nc("full")
    in_maps = make_inputs(q, k, y, cfg)
    res = run_bass_kernel_spmd(nc, in_maps, core_ids=list(range(NCORES)),
                               trace=trace)
    total = np.sum([res.results[r]["out"][0, 0] for r in range(NCORES)],
                   dtype=np.float64)
    out = np.asarray(total / cfg.N, dtype=np.float32)
    if trace:
        kernel.last_results = res
    return out
